# revision 11
# baseline (speedup 1.0000x reference)
"""GAT layer (PyG-style, add_self_loops=True) on 8 Trainium2 NeuronCores.

Strategy: partition destination nodes (and their incident edges) across the 8
cores; each core owns a contiguous range of 6250 dst nodes (49 windows of 128).

No projection table. Per window of 128 dst nodes, two transposed dma_gathers
(lo/hi halves of the node range, int16-index limit) pull the raw 256-byte x
rows of all incident edges' sources straight out of HBM, TRANSPOSED at u16
granularity: the host pre-interleaves each x row's bytes as
[hi16(x_0)..hi16(x_63) | lo16(x_0)..lo16(x_63)], so gather partitions 0:64
hold truncated-bf16 features and serve directly as the matmul lhsT. Each
128-edge subtile then computes h|a_src = x_src @ [W | W@att_src^T] as ONE bf16
matmul (f32 PSUM), so there is no replicated 50k-row projection pass and no
38 MB table write at all.

Per-edge a_dst: one-hot(edge->dst slot) built on DVE in bf16, PE-transposed,
then a tiny matmul against the window's a_dst vector (phase-1b: 49 small
matmuls over the core's own dst shard). exp(leaky_relu(score)) on DVE+Act.
Segment-sum of [e*h | e] via PSUM-accumulated one-hot matmuls; final
out = acc/(denom+eps) + bias.  Softmax max-subtraction is skipped
(shift-invariant; scores are O(1) so exp cannot overflow).

Pad edges gather row 0 (finite) and carry dst-slot sentinel 500 => their
one-hot row is all zero, so they contribute to nothing. No dummy rows.

Host does only index/byte-space work (self-loop append, dst sort, windowing,
padding, int16 index wrapping, u16 byte interleave of x, x transpose/slice).
"""

import math

import numpy as np

N = 50000
IN_DIM = 64
H = 4
D = 64
HD = H * D  # 256
WCOLS = HD + H  # 260: per-edge matmul output h | a_src
NEG_SLOPE = 0.2
EPS = 1e-16
SENT = 500.0  # dst-slot sentinel for pad edges (one-hot row all zero)

NCORES = 8
NPC = N // NCORES  # 6250 dst nodes per core
NWIN = math.ceil(NPC / 128)  # 49 windows
WROWS = NWIN * 128  # 6272
SPLIT = 25088  # lo/hi x-table split (int16 gather index limit)
SUBS = 2  # edge subtiles per chunk (PSUM-bank budget)
CSTRIDE = 512  # psum cols per subtile slot (bank-aligned; 260 used)
ADX0 = 260  # col in each subtile's psum slot where a_dst-per-edge lands
# (contiguous with a_src at 256:260 so score = reduce_sum over the pair
#  reads PSUM with a single input AP -- DVE allows only one PSUM operand)

LAST_RESULTS = None  # BassKernelResults of the most recent run (for test.py)


def _wrap_idx(ids):
    """[n] int -> dma_gather wrapped layout [128, n/16] int16
    (idx i at [i%16, i//16], replicated across the 8 Q7 core groups)."""
    n = len(ids)
    w16 = ids.reshape(n // 16, 16).T.astype(np.int16)  # [16, n/16]
    return np.tile(w16, (8, 1))


def _interleave_x(x):
    """[N,64] f32 -> [N,128] u16 rows [hi16(x_0..63) | lo16(x_0..63)].
    After the u16-granularity transposed gather, partitions 0:64 hold the
    high halves = truncated-bf16 feature values."""
    xu = np.ascontiguousarray(x).view(np.uint16).reshape(-1, 64, 2)
    return np.ascontiguousarray(np.concatenate([xu[:, :, 1], xu[:, :, 0]], axis=1))


def _prep_host(edge_index):
    """Returns ilow  int16 [NCORES, NWIN, 128, KL*8]
               ihigh int16 [NCORES, NWIN, 128, KH*8]
               dstrel f32  [NCORES, NWIN, 128, KL+KH]  (slot or SENT)
               (KL, KH)"""
    src = np.concatenate([edge_index[0], np.arange(N, dtype=np.int64)]).astype(np.int64)
    dst = np.concatenate([edge_index[1], np.arange(N, dtype=np.int64)]).astype(np.int64)
    order = np.argsort(dst, kind="stable")
    src = src[order].astype(np.int32)
    dst = dst[order].astype(np.int32)

    bounds = [c * NPC + w * 128 for c in range(NCORES) for w in range(NWIN)]
    bounds.append(N)
    cuts = np.searchsorted(dst, np.asarray(bounds))

    lo_counts = np.zeros(NCORES * NWIN, np.int64)
    hi_counts = np.zeros(NCORES * NWIN, np.int64)
    for b in range(NCORES * NWIN):
        s = src[cuts[b] : cuts[b + 1]]
        lo_counts[b] = int((s < SPLIT).sum())
        hi_counts[b] = len(s) - lo_counts[b]
    KL = max(1, math.ceil(lo_counts.max() / 128))
    KH = max(1, math.ceil(hi_counts.max() / 128))
    kj = KL + KH

    ilow = np.zeros((NCORES, NWIN, KL * 128), np.int32)  # pad -> row 0 (finite)
    ihigh = np.zeros((NCORES, NWIN, KH * 128), np.int32)
    dstrel = np.full((NCORES, NWIN, 128, kj), SENT, np.float32)
    for c in range(NCORES):
        base = c * NPC
        for w in range(NWIN):
            b = c * NWIN + w
            s = src[cuts[b] : cuts[b + 1]]
            d = dst[cuts[b] : cuts[b + 1]] - base - w * 128
            m = s < SPLIT
            slo, dlo = s[m], d[m]
            shi, dhi = s[~m] - SPLIT, d[~m]
            # ascending source rows => HBM page locality in the gather
            o = np.argsort(slo, kind="stable")
            slo, dlo = slo[o], dlo[o]
            o = np.argsort(shi, kind="stable")
            shi, dhi = shi[o], dhi[o]
            ilow[c, w, : len(slo)] = slo
            ihigh[c, w, : len(shi)] = shi
            i = np.arange(len(slo))
            dstrel[c, w, i % 128, i // 128] = dlo
            i = np.arange(len(shi))
            dstrel[c, w, i % 128, KL + i // 128] = dhi
    ilow_w = np.zeros((NCORES, NWIN, 128, KL * 8), np.int16)
    ihigh_w = np.zeros((NCORES, NWIN, 128, KH * 8), np.int16)
    for c in range(NCORES):
        for w in range(NWIN):
            ilow_w[c, w] = _wrap_idx(ilow[c, w])
            ihigh_w[c, w] = _wrap_idx(ihigh[c, w])
    return ilow_w, ihigh_w, dstrel, KL, KH


def _build_program(KL, KH, ablate="full"):
    import concourse.bass as bass
    import concourse.bacc as bacc
    import concourse.tile as tile
    from concourse import mybir
    from concourse.masks import make_identity

    f32 = mybir.dt.float32
    bf16 = mybir.dt.bfloat16
    u16 = mybir.dt.uint16
    i16 = mybir.dt.int16
    i32 = mybir.dt.int32
    kj = KL + KH

    nc = bacc.Bacc(None, target_bir_lowering=False)

    xlo_d = nc.dram_tensor("x_lo", [SPLIT, 128], u16, kind="ExternalInput")
    xhi_d = nc.dram_tensor("x_hi", [N - SPLIT, 128], u16, kind="ExternalInput")
    xdT_d = nc.dram_tensor("xdstT", [IN_DIM, WROWS], f32, kind="ExternalInput")
    W_d = nc.dram_tensor("W", [IN_DIM, HD], f32, kind="ExternalInput")
    asrc_d = nc.dram_tensor("att_src", [1, HD], f32, kind="ExternalInput")
    adst_d = nc.dram_tensor("att_dst", [1, HD], f32, kind="ExternalInput")
    bias_d = nc.dram_tensor("bias", [1, HD], f32, kind="ExternalInput")
    il_d = nc.dram_tensor("ilow", [NWIN, 128, KL * 8], i16, kind="ExternalInput")
    ih_d = nc.dram_tensor("ihigh", [NWIN, 128, KH * 8], i16, kind="ExternalInput")
    drel_d = nc.dram_tensor("dstrel", [NWIN, 128, kj], f32, kind="ExternalInput")
    out_d = nc.dram_tensor("out", [WROWS, HD], f32, kind="ExternalOutput")

    X = mybir.AxisListType.X
    EQ = mybir.AluOpType.is_equal
    MULT = mybir.AluOpType.mult
    MAX = mybir.AluOpType.max

    with tile.TileContext(nc) as tc:
        with tc.tile_pool(name="const", bufs=1) as cpool:
            spsum_cm = tc.tile_pool(name="setup_psum", bufs=1, space="PSUM")
            spsum = spsum_cm.__enter__()
            ones = cpool.tile([1, 128], f32)
            nc.vector.memset(ones[:], 1.0)
            ident_f = cpool.tile([128, 128], f32)
            make_identity(nc, ident_f[:])
            ident = cpool.tile([128, 128], bf16)
            nc.vector.tensor_copy(ident[:], ident_f[:])
            iota_i = cpool.tile([128, 128], i32)
            nc.gpsimd.iota(iota_i[:], pattern=[[1, 128]], base=0, channel_multiplier=0)
            iota_f = cpool.tile([128, 128], bf16)
            nc.vector.tensor_copy(iota_f[:], iota_i[:])

            # WA = [W | Wsrc], Wsrc[k,h] = sum_d W[k,h*D+d]*att_src[h,d];
            # Wdst likewise (kept f32 for the per-window a_dst matmuls).
            wa_tmp = cpool.tile([IN_DIM, WCOLS], f32)
            nc.vector.memset(wa_tmp[:], 0.0)
            nc.sync.dma_start(wa_tmp[:, 0:HD], W_d[:, :])
            wdst = cpool.tile([IN_DIM, H], f32)
            att_s_raw = cpool.tile([1, HD], f32)
            nc.sync.dma_start(att_s_raw[:], asrc_d[:, :])
            att_t_raw = cpool.tile([1, HD], f32)
            nc.sync.dma_start(att_t_raw[:], adst_d[:, :])
            att_s = cpool.tile([1, HD], f32)
            nc.vector.tensor_copy(att_s[:], att_s_raw[:])
            att_t = cpool.tile([1, HD], f32)
            nc.vector.tensor_copy(att_t[:], att_t_raw[:])
            for att_tile, dst_ap in ((att_s, wa_tmp[:, HD : HD + H]), (att_t, wdst[:, :])):
                attb = spsum.tile([IN_DIM, HD], f32, tag="attb")
                nc.tensor.matmul(
                    attb[:], lhsT=ones[:1, 0:IN_DIM], rhs=att_tile[:],
                    start=True, stop=True,
                )
                tmp = cpool.tile([IN_DIM, HD], f32, tag="tmp")
                nc.vector.tensor_mul(tmp[:], wa_tmp[:, 0:HD], attb[:])
                nc.vector.reduce_sum(
                    out=dst_ap,
                    in_=tmp[:].rearrange("k (h d) -> k h d", d=D),
                    axis=X,
                )
            WAb = cpool.tile([IN_DIM, WCOLS], bf16)
            nc.vector.tensor_copy(WAb[:], wa_tmp[:])

            bias_raw = cpool.tile([1, HD], f32)
            nc.sync.dma_start(bias_raw[:], bias_d[:, :])
            bias_sb = cpool.tile([1, HD], f32)
            nc.vector.tensor_copy(bias_sb[:], bias_raw[:])
            bb = spsum.tile([128, HD], f32)
            nc.tensor.matmul(bb[:], lhsT=ones[:1, :], rhs=bias_sb[:], start=True, stop=True)
            bias_bc = cpool.tile([128, HD], f32)
            nc.scalar.copy(bias_bc[:], bb[:])

            # phase 1b: a_dst for the core's own dst shard, bf16 [128, NWIN*H]
            adst_all = cpool.tile([128, NWIN * H], bf16)

            # preloaded per-window index data (batched DMAs -- per-DMA fixed
            # cost on HW is ~2us, so per-window loads are expensive)
            il_all = cpool.tile([128, NWIN * KL * 8], i16)
            nc.sync.dma_start(
                il_all[:].rearrange("p (w k) -> p w k", w=NWIN),
                il_d[:, :, :].rearrange("w p k -> p w k"),
            )
            ih_all = cpool.tile([128, NWIN * KH * 8], i16)
            nc.sync.dma_start(
                ih_all[:].rearrange("p (w k) -> p w k", w=NWIN),
                ih_d[:, :, :].rearrange("w p k -> p w k"),
            )
            drel_raw = cpool.tile([128, NWIN * kj], f32)
            nc.sync.dma_start(
                drel_raw[:].rearrange("p (w k) -> p w k", w=NWIN),
                drel_d[:, :, :].rearrange("w p k -> p w k"),
            )
            drel_all = cpool.tile([128, NWIN * kj], bf16)
            nc.vector.tensor_copy(drel_all[:], drel_raw[:])

            spsum_cm.__exit__(None, None, None)  # free setup PSUM banks

            P1B = 8  # windows per psum batch
            with (
                tc.tile_pool(name="p1", bufs=1) as p1,
                tc.tile_pool(name="p1ps", bufs=2, space="PSUM") as p1ps,
            ):
                xd = p1.tile([IN_DIM, WROWS], f32, tag="xdr")
                nc.sync.dma_start(xd[:], xdT_d[:, :])
                for w0 in range(0, NWIN, P1B):
                    nb = min(P1B, NWIN - w0)
                    adp = p1ps.tile([128, P1B * H], f32, tag="adp")
                    for k in range(nb):
                        w = w0 + k
                        nc.tensor.matmul(
                            adp[:, k * H : (k + 1) * H],
                            lhsT=xd[:, w * 128 : (w + 1) * 128],
                            rhs=wdst[:], start=True, stop=True,
                        )
                    nc.vector.tensor_copy(
                        adst_all[:, w0 * H : (w0 + nb) * H], adp[:, 0 : nb * H]
                    )

            # ---------------- main loop: per-window aggregation ----------------
            if ablate == "p1":
                with tc.tile_pool(name="fin0", bufs=1) as f0:
                    zo = f0.tile([128, HD], f32)
                    nc.vector.memset(zo[:], 0.0)
                    for w in range(NWIN):
                        nc.sync.dma_start(out_d[w * 128 : (w + 1) * 128, :], zo[:])
            nch = math.ceil(kj / SUBS)
            OB = 7  # windows per output-write batch (49 = 7*7)
            with (
                tc.tile_pool(name="gat", bufs=3) as gpool,
                tc.tile_pool(name="wrk", bufs=4) as wpool,
                tc.tile_pool(name="fin", bufs=2) as fpool,
                tc.tile_pool(name="hps", bufs=2, space="PSUM") as hpool,
                tc.tile_pool(name="sps", bufs=2, space="PSUM") as spool,
                tc.tile_pool(name="acc", bufs=2, space="PSUM") as apool,
            ):
                for g0 in range(0, NWIN if ablate != "p1" else 0, OB):
                  gnb = min(OB, NWIN - g0)
                  outb = fpool.tile([128, OB * HD], f32, tag="outb")
                  for k in range(gnb):
                    w = g0 + k
                    g = gpool.tile([128, kj * 128], u16, tag="g")
                    gv = g[:].unsqueeze(1)  # [128, 1, kj*128]
                    nc.gpsimd.dma_gather(
                        out_ap=gv[:, :, 0 : KL * 128], in_ap=xlo_d[:, :],
                        idxs_ap=il_all[:, w * KL * 8 : (w + 1) * KL * 8],
                        num_idxs=KL * 128, num_idxs_reg=KL * 128,
                        elem_size=128, transpose=True, single_packet=False,
                    )
                    nc.gpsimd.dma_gather(
                        out_ap=gv[:, :, KL * 128 : kj * 128], in_ap=xhi_d[:, :],
                        idxs_ap=ih_all[:, w * KH * 8 : (w + 1) * KH * 8],
                        num_idxs=KH * 128, num_idxs_reg=KH * 128,
                        elem_size=128, transpose=True, single_packet=False,
                    )
                    gb = g[:].bitcast(bf16)
                    adw = adst_all[:, w * H : (w + 1) * H]
                    drel_b = drel_all[:, w * kj : (w + 1) * kj]

                    if ablate == "p1g":
                        nc.vector.memset(outb[:, k * HD : (k + 1) * HD], 0.0)
                        nc.vector.tensor_copy(
                            outb[0:IN_DIM, k * HD : k * HD + HD],
                            gb[0:IN_DIM, 0:HD],
                        )
                        if k == gnb - 1:
                            nc.sync.dma_start(
                                out_d[g0 * 128 : (g0 + gnb) * 128, :].rearrange(
                                    "(k p) c -> p k c", p=128
                                ),
                                outb[:, 0 : gnb * HD].rearrange(
                                    "p (k c) -> p k c", c=HD
                                ),
                            )
                        continue

                    accdns = apool.tile([128, WCOLS], f32, tag="accdns")
                    for ch in range(nch):
                        s0 = ch * SUBS
                        ns = min(SUBS, kj - s0)
                        # per-edge projection: hps[:, s*512 : s*512+260] =
                        #   [h | a_src] of subtile s (bf16 matmul, f32 psum)
                        hps = hpool.tile([128, SUBS * CSTRIDE], f32, tag="hps")
                        for s in range(ns):
                            nc.tensor.matmul(
                                hps[:, s * CSTRIDE : s * CSTRIDE + WCOLS],
                                lhsT=gb[0:IN_DIM, (s0 + s) * 128 : (s0 + s + 1) * 128],
                                rhs=WAb[:],
                                start=True, stop=True,
                            )
                        # one-hot[e, s, p] = (dstrel[e, s] == p), bf16
                        oh = wpool.tile([128, SUBS * 128], bf16, tag="oh")
                        nc.vector.tensor_tensor(
                            out=oh[:, 0 : ns * 128].rearrange("p (s e) -> p s e", s=ns),
                            in0=drel_b[:, s0 : s0 + ns].unsqueeze(-1).to_broadcast(
                                [128, ns, 128]
                            ),
                            in1=iota_f[:].unsqueeze(1).to_broadcast([128, ns, 128]),
                            op=EQ,
                        )
                        # transposed one-hot (for a_dst expansion) + a_dst matmul
                        # (adx lands in unused hps columns ADX0:ADX0+H per slot
                        # -- no extra PSUM bank needed)
                        ohts = spool.tile([128, SUBS * 128], bf16, tag="ohts")
                        for s in range(ns):
                            nc.tensor.transpose(
                                ohts[:, s * 128 : (s + 1) * 128],
                                oh[:, s * 128 : (s + 1) * 128],
                                ident[:],
                            )
                        ohT = wpool.tile([128, SUBS * 128], bf16, tag="ohT")
                        nc.scalar.copy(ohT[:, 0 : ns * 128], ohts[:, 0 : ns * 128])
                        for s in range(ns):
                            nc.tensor.matmul(
                                hps[:, s * CSTRIDE + ADX0 : s * CSTRIDE + ADX0 + H],
                                lhsT=ohT[:, s * 128 : (s + 1) * 128],
                                rhs=adw,
                                start=True, stop=True,
                            )
                        # score = a_src + a_dst -> leaky_relu -> exp (bf16)
                        hv = hps[:].rearrange("p (s c) -> p s c", c=CSTRIDE)
                        sc = wpool.tile([128, SUBS * H], f32, tag="sc")
                        nc.vector.reduce_sum(
                            out=sc[:, 0 : ns * H].rearrange("p (s h) -> p s h", h=H),
                            in_=hv[:, 0:ns, HD : HD + 2 * H].rearrange(
                                "p s (a h) -> p s h a", a=2
                            ),
                            axis=X,
                        )
                        lr = wpool.tile([128, SUBS * H], f32, tag="lr")
                        nc.vector.scalar_tensor_tensor(
                            out=lr[:, 0 : ns * H], in0=sc[:, 0 : ns * H],
                            scalar=NEG_SLOPE, in1=sc[:, 0 : ns * H],
                            op0=MULT, op1=MAX,
                        )
                        ex = wpool.tile([128, SUBS * H], bf16, tag="ex")
                        nc.scalar.activation(
                            ex[:, 0 : ns * H], lr[:, 0 : ns * H],
                            mybir.ActivationFunctionType.Exp,
                        )
                        # msg[:, s, 0:HD] = h_s * e ; msg[:, s, HD:HD+H] = e
                        # (mul reads ex directly; the e-column copy runs in
                        # parallel, off the critical path)
                        msg = wpool.tile([128, SUBS * WCOLS], bf16, tag="msg")
                        mv = msg[:].rearrange("p (s c) -> p s c", s=SUBS)
                        nc.vector.tensor_copy(
                            mv[:, 0:ns, HD : HD + H],
                            ex[:, 0 : ns * H].rearrange("p (s h) -> p s h", h=H),
                        )
                        nc.vector.tensor_mul(
                            mv[:, 0:ns, 0:HD].rearrange("p s (h d) -> p s h d", d=D),
                            hv[:, 0:ns, 0:HD].rearrange("p s (h d) -> p s h d", d=D),
                            ex[:, 0 : ns * H]
                            .rearrange("p (s h) -> p s h", h=H)
                            .unsqueeze(-1)
                            .to_broadcast([128, ns, H, D]),
                        )
                        # accumulate [sum(e*h) | sum(e)] over the window
                        for s in range(ns):
                            q = s0 + s
                            nc.tensor.matmul(
                                accdns[:],
                                lhsT=oh[:, s * 128 : (s + 1) * 128],
                                rhs=mv[:, s, :],
                                start=(q == 0),
                                stop=(q == kj - 1),
                            )
                    # finalize: out = acc / (dns + eps) + bias
                    dnse = fpool.tile([128, H], f32, tag="dnse")
                    nc.vector.tensor_scalar_add(dnse[:], accdns[:, HD : HD + H], EPS)
                    dnr = fpool.tile([128, H], f32, tag="dnr")
                    nc.vector.reciprocal(dnr[:], dnse[:])
                    outw = outb[:, k * HD : (k + 1) * HD]
                    nc.vector.tensor_mul(
                        outw.rearrange("p (h d) -> p h d", d=D),
                        accdns[:, 0:HD].rearrange("p (h d) -> p h d", d=D),
                        dnr[:].unsqueeze(-1).to_broadcast([128, H, D]),
                    )
                    nc.vector.tensor_add(outw, outw, bias_bc[:])
                    if k == gnb - 1:
                        nc.sync.dma_start(
                            out_d[g0 * 128 : (g0 + gnb) * 128, :].rearrange(
                                "(k p) c -> p k c", p=128
                            ),
                            outb[:, 0 : gnb * HD].rearrange("p (k c) -> p k c", c=HD),
                        )
    nc.compile()
    # compile()'s late passes (act-table loads, hostgen rebases) can leave
    # >1-wait instructions behind; one more split pass clears them (the TRN2
    # ISA allows a single sem wait per compute instruction).
    nc.generate_event_semaphores()
    return nc


def _stage_inputs(x, W, att_src, att_dst, bias, ilow, ihigh, dstrel):
    x = np.asarray(x, dtype=np.float32)
    x_il = _interleave_x(x)
    x_lo = np.ascontiguousarray(x_il[:SPLIT])
    x_hi = np.ascontiguousarray(x_il[SPLIT:])
    asrc_row = np.ascontiguousarray(np.asarray(att_src, np.float32).reshape(1, HD))
    adst_row = np.ascontiguousarray(np.asarray(att_dst, np.float32).reshape(1, HD))
    bias_row = np.ascontiguousarray(np.asarray(bias, np.float32).reshape(1, HD))
    in_maps = []
    for c in range(NCORES):
        xdT = np.zeros((IN_DIM, WROWS), dtype=np.float32)
        xdT[:, :NPC] = x[c * NPC : (c + 1) * NPC].T
        in_maps.append(
            {
                "x_lo": x_lo,
                "x_hi": x_hi,
                "xdstT": np.ascontiguousarray(xdT),
                "W": np.asarray(W, np.float32),
                "att_src": asrc_row,
                "att_dst": adst_row,
                "bias": bias_row,
                "ilow": np.ascontiguousarray(ilow[c]),
                "ihigh": np.ascontiguousarray(ihigh[c]),
                "dstrel": np.ascontiguousarray(dstrel[c]),
            }
        )
    return in_maps


def kernel(x, edge_index, W, att_src, att_dst, bias):
    global LAST_RESULTS
    from concourse.bass_utils import run_bass_kernel_spmd

    edge_index = np.asarray(edge_index)
    ilow, ihigh, dstrel, KL, KH = _prep_host(edge_index)
    nc = _build_program(KL, KH)
    in_maps = _stage_inputs(x, W, att_src, att_dst, bias, ilow, ihigh, dstrel)

    res = run_bass_kernel_spmd(nc, in_maps, list(range(NCORES)))
    LAST_RESULTS = res

    out = np.empty((N, HD), dtype=np.float32)
    for c in range(NCORES):
        out[c * NPC : (c + 1) * NPC] = res.results[c]["out"][:NPC]
    return out


# revision 14
# speedup vs baseline: 1.1441x; 1.1441x over previous
"""GAT layer (PyG-style, add_self_loops=True) on 8 Trainium2 NeuronCores.

Strategy: partition destination nodes (and their incident edges) across the 8
cores; each core owns a contiguous range of 6250 dst nodes (49 windows of 128).

No projection table. Per window of 128 dst nodes, two transposed dma_gathers
(lo/hi halves of the node range, int16-index limit) pull the raw 256-byte x
rows of all incident edges' sources straight out of HBM, TRANSPOSED at u16
granularity: the host pre-interleaves each x row's bytes as
[hi16(x_0)..hi16(x_63) | lo16(x_0)..lo16(x_63)], so gather partitions 0:64
hold truncated-bf16 features and serve directly as the matmul lhsT. Each
128-edge subtile then computes h|a_src = x_src @ [W | W@att_src^T] as ONE bf16
matmul (f32 PSUM), so there is no replicated 50k-row projection pass and no
38 MB table write at all.

Per-edge a_dst: one-hot(edge->dst slot) built on DVE in bf16, PE-transposed,
then a tiny matmul against the window's a_dst vector (phase-1b: 49 small
matmuls over the core's own dst shard). exp(leaky_relu(score)) on DVE+Act.
Segment-sum of [e*h | e] via PSUM-accumulated one-hot matmuls; final
out = acc/(denom+eps) + bias.  Softmax max-subtraction is skipped
(shift-invariant; scores are O(1) so exp cannot overflow).

Pad edges gather row 0 (finite) and carry dst-slot sentinel 500 => their
one-hot row is all zero, so they contribute to nothing. No dummy rows.

Host does only index/byte-space work (self-loop append, dst sort, windowing,
padding, int16 index wrapping, u16 byte interleave of x, x transpose/slice).
"""

import math

import numpy as np

N = 50000
IN_DIM = 64
H = 4
D = 64
HD = H * D  # 256
WCOLS = HD + H  # 260: per-edge matmul output h | a_src
NEG_SLOPE = 0.2
EPS = 1e-16
SENT = 500.0  # dst-slot sentinel for pad edges (one-hot row all zero)

NCORES = 8
NPC = N // NCORES  # 6250 dst nodes per core
NWIN = math.ceil(NPC / 128)  # 49 windows
WROWS = NWIN * 128  # 6272
SPLIT = 25088  # lo/hi x-table split (int16 gather index limit)
SUBS = 2  # edge subtiles per chunk (PSUM-bank budget)
CSTRIDE = 512  # psum cols per subtile slot (bank-aligned; 260 used)
ADX0 = 260  # col in each subtile's psum slot where a_dst-per-edge lands
# (contiguous with a_src at 256:260 so score = reduce_sum over the pair
#  reads PSUM with a single input AP -- DVE allows only one PSUM operand)

LAST_RESULTS = None  # BassKernelResults of the most recent run (for test.py)


def _wrap_idx(ids):
    """[n] int -> dma_gather wrapped layout [128, n/16] int16
    (idx i at [i%16, i//16], replicated across the 8 Q7 core groups)."""
    n = len(ids)
    w16 = ids.reshape(n // 16, 16).T.astype(np.int16)  # [16, n/16]
    return np.tile(w16, (8, 1))


def _interleave_x(x):
    """[N,64] f32 -> [N,128] u16 rows [hi16(x_0..63) | lo16(x_0..63)].
    After the u16-granularity transposed gather, partitions 0:64 hold the
    high halves = truncated-bf16 feature values."""
    xu = np.ascontiguousarray(x).view(np.uint16).reshape(-1, 64, 2)
    return np.ascontiguousarray(np.concatenate([xu[:, :, 1], xu[:, :, 0]], axis=1))


def _prep_host(edge_index):
    """Returns ilow  int16 [NCORES, NWIN, 128, KL*8]
               ihigh int16 [NCORES, NWIN, 128, KH*8]
               dstrel f32  [NCORES, NWIN, 128, KL+KH]  (slot or SENT)
               (KL, KH)"""
    src = np.concatenate([edge_index[0], np.arange(N, dtype=np.int64)]).astype(np.int64)
    dst = np.concatenate([edge_index[1], np.arange(N, dtype=np.int64)]).astype(np.int64)
    order = np.argsort(dst, kind="stable")
    src = src[order].astype(np.int32)
    dst = dst[order].astype(np.int32)

    bounds = [c * NPC + w * 128 for c in range(NCORES) for w in range(NWIN)]
    bounds.append(N)
    cuts = np.searchsorted(dst, np.asarray(bounds))

    lo_counts = np.zeros(NCORES * NWIN, np.int64)
    hi_counts = np.zeros(NCORES * NWIN, np.int64)
    for b in range(NCORES * NWIN):
        s = src[cuts[b] : cuts[b + 1]]
        lo_counts[b] = int((s < SPLIT).sum())
        hi_counts[b] = len(s) - lo_counts[b]
    KL = max(1, math.ceil(lo_counts.max() / 128))
    KH = max(1, math.ceil(hi_counts.max() / 128))
    kj = KL + KH

    ilow = np.zeros((NCORES, NWIN, KL * 128), np.int32)  # pad -> row 0 (finite)
    ihigh = np.zeros((NCORES, NWIN, KH * 128), np.int32)
    dstrel = np.full((NCORES, NWIN, 128, kj), SENT, np.float32)
    for c in range(NCORES):
        base = c * NPC
        for w in range(NWIN):
            b = c * NWIN + w
            s = src[cuts[b] : cuts[b + 1]]
            d = dst[cuts[b] : cuts[b + 1]] - base - w * 128
            m = s < SPLIT
            slo, dlo = s[m], d[m]
            shi, dhi = s[~m] - SPLIT, d[~m]
            # ascending source rows => HBM page locality in the gather
            o = np.argsort(slo, kind="stable")
            slo, dlo = slo[o], dlo[o]
            o = np.argsort(shi, kind="stable")
            shi, dhi = shi[o], dhi[o]
            ilow[c, w, : len(slo)] = slo
            ihigh[c, w, : len(shi)] = shi
            i = np.arange(len(slo))
            dstrel[c, w, i % 128, i // 128] = dlo
            i = np.arange(len(shi))
            dstrel[c, w, i % 128, KL + i // 128] = dhi
    ilow_w = np.zeros((NCORES, NWIN, 128, KL * 8), np.int16)
    ihigh_w = np.zeros((NCORES, NWIN, 128, KH * 8), np.int16)
    for c in range(NCORES):
        for w in range(NWIN):
            ilow_w[c, w] = _wrap_idx(ilow[c, w])
            ihigh_w[c, w] = _wrap_idx(ihigh[c, w])
    return ilow_w, ihigh_w, dstrel, KL, KH


def _build_program(KL, KH, ablate="full"):
    import concourse.bass as bass
    import concourse.bacc as bacc
    import concourse.tile as tile
    from concourse import mybir
    from concourse.masks import make_identity

    f32 = mybir.dt.float32
    bf16 = mybir.dt.bfloat16
    u16 = mybir.dt.uint16
    i16 = mybir.dt.int16
    i32 = mybir.dt.int32
    kj = KL + KH

    nc = bacc.Bacc(None, target_bir_lowering=False)

    xlo_d = nc.dram_tensor("x_lo", [SPLIT, 128], u16, kind="ExternalInput")
    xhi_d = nc.dram_tensor("x_hi", [N - SPLIT, 128], u16, kind="ExternalInput")
    xdT_d = nc.dram_tensor("xdstT", [IN_DIM, WROWS], f32, kind="ExternalInput")
    W_d = nc.dram_tensor("W", [IN_DIM, HD], f32, kind="ExternalInput")
    asrc_d = nc.dram_tensor("att_src", [1, HD], f32, kind="ExternalInput")
    adst_d = nc.dram_tensor("att_dst", [1, HD], f32, kind="ExternalInput")
    bias_d = nc.dram_tensor("bias", [1, HD], f32, kind="ExternalInput")
    # partition-major on host so the one-shot preload DMA is contiguous per
    # partition (128 large descriptors, not NWIN*128 small ones)
    il_d = nc.dram_tensor("ilow", [128, NWIN * KL * 8], i16, kind="ExternalInput")
    ih_d = nc.dram_tensor("ihigh", [128, NWIN * KH * 8], i16, kind="ExternalInput")
    drel_d = nc.dram_tensor("dstrel", [128, NWIN * kj], f32, kind="ExternalInput")
    out_d = nc.dram_tensor("out", [WROWS, HD], f32, kind="ExternalOutput")

    X = mybir.AxisListType.X
    EQ = mybir.AluOpType.is_equal
    MULT = mybir.AluOpType.mult
    MAX = mybir.AluOpType.max

    with tile.TileContext(nc) as tc:
        with tc.tile_pool(name="const", bufs=1) as cpool:
            spsum_cm = tc.tile_pool(name="setup_psum", bufs=1, space="PSUM")
            spsum = spsum_cm.__enter__()
            ones = cpool.tile([1, 128], f32)
            nc.vector.memset(ones[:], 1.0)
            ident_f = cpool.tile([128, 128], f32)
            make_identity(nc, ident_f[:])
            ident = cpool.tile([128, 128], bf16)
            nc.vector.tensor_copy(ident[:], ident_f[:])
            iota_i = cpool.tile([128, 128], i32)
            nc.gpsimd.iota(iota_i[:], pattern=[[1, 128]], base=0, channel_multiplier=0)
            iota_f = cpool.tile([128, 128], bf16)
            nc.vector.tensor_copy(iota_f[:], iota_i[:])

            # WA = [W | Wsrc], Wsrc[k,h] = sum_d W[k,h*D+d]*att_src[h,d];
            # Wdst likewise (kept f32 for the per-window a_dst matmuls).
            wa_tmp = cpool.tile([IN_DIM, WCOLS], f32)
            nc.vector.memset(wa_tmp[:], 0.0)
            nc.sync.dma_start(wa_tmp[:, 0:HD], W_d[:, :])
            wdst = cpool.tile([IN_DIM, H], f32)
            att_s_raw = cpool.tile([1, HD], f32)
            nc.sync.dma_start(att_s_raw[:], asrc_d[:, :])
            att_t_raw = cpool.tile([1, HD], f32)
            nc.sync.dma_start(att_t_raw[:], adst_d[:, :])
            att_s = cpool.tile([1, HD], f32)
            nc.vector.tensor_copy(att_s[:], att_s_raw[:])
            att_t = cpool.tile([1, HD], f32)
            nc.vector.tensor_copy(att_t[:], att_t_raw[:])
            for att_tile, dst_ap in ((att_s, wa_tmp[:, HD : HD + H]), (att_t, wdst[:, :])):
                attb = spsum.tile([IN_DIM, HD], f32, tag="attb")
                nc.tensor.matmul(
                    attb[:], lhsT=ones[:1, 0:IN_DIM], rhs=att_tile[:],
                    start=True, stop=True,
                )
                tmp = cpool.tile([IN_DIM, HD], f32, tag="tmp")
                nc.vector.tensor_mul(tmp[:], wa_tmp[:, 0:HD], attb[:])
                nc.vector.reduce_sum(
                    out=dst_ap,
                    in_=tmp[:].rearrange("k (h d) -> k h d", d=D),
                    axis=X,
                )
            WAb = cpool.tile([IN_DIM, WCOLS], bf16)
            nc.vector.tensor_copy(WAb[:], wa_tmp[:])

            bias_raw = cpool.tile([1, HD], f32)
            nc.sync.dma_start(bias_raw[:], bias_d[:, :])
            bias_sb = cpool.tile([1, HD], f32)
            nc.vector.tensor_copy(bias_sb[:], bias_raw[:])
            bb = spsum.tile([128, HD], f32)
            nc.tensor.matmul(bb[:], lhsT=ones[:1, :], rhs=bias_sb[:], start=True, stop=True)
            bias_bc = cpool.tile([128, HD], f32)
            nc.scalar.copy(bias_bc[:], bb[:])

            # phase 1b: a_dst for the core's own dst shard, bf16 [128, NWIN*H]
            adst_all = cpool.tile([128, NWIN * H], bf16)

            # preloaded per-window index data (batched DMAs -- per-DMA fixed
            # cost on HW is ~2us, so per-window loads are expensive)
            il_all = cpool.tile([128, NWIN * KL * 8], i16)
            nc.sync.dma_start(il_all[:], il_d[:, :])
            ih_all = cpool.tile([128, NWIN * KH * 8], i16)
            nc.sync.dma_start(ih_all[:], ih_d[:, :])
            drel_raw = cpool.tile([128, NWIN * kj], f32)
            nc.sync.dma_start(drel_raw[:], drel_d[:, :])
            drel_all = cpool.tile([128, NWIN * kj], bf16)
            nc.vector.tensor_copy(drel_all[:], drel_raw[:])

            spsum_cm.__exit__(None, None, None)  # free setup PSUM banks

            P1B = 8  # windows per psum batch
            with (
                tc.tile_pool(name="p1", bufs=1) as p1,
                tc.tile_pool(name="p1ps", bufs=2, space="PSUM") as p1ps,
            ):
                xd = p1.tile([IN_DIM, WROWS], f32, tag="xdr")
                nc.sync.dma_start(xd[:], xdT_d[:, :])
                for w0 in range(0, NWIN, P1B):
                    nb = min(P1B, NWIN - w0)
                    adp = p1ps.tile([128, P1B * H], f32, tag="adp")
                    for k in range(nb):
                        w = w0 + k
                        nc.tensor.matmul(
                            adp[:, k * H : (k + 1) * H],
                            lhsT=xd[:, w * 128 : (w + 1) * 128],
                            rhs=wdst[:], start=True, stop=True,
                        )
                    nc.vector.tensor_copy(
                        adst_all[:, w0 * H : (w0 + nb) * H], adp[:, 0 : nb * H]
                    )

            # ---------------- main loop: per-window aggregation ----------------
            if ablate == "p1":
                with tc.tile_pool(name="fin0", bufs=1) as f0:
                    zo = f0.tile([128, HD], f32)
                    nc.vector.memset(zo[:], 0.0)
                    for w in range(NWIN):
                        nc.sync.dma_start(out_d[w * 128 : (w + 1) * 128, :], zo[:])
            nch = math.ceil(kj / SUBS)
            OB = 7  # windows per output-write batch (49 = 7*7)
            with (
                tc.tile_pool(name="gat", bufs=3) as gpool,
                tc.tile_pool(name="wrk", bufs=4) as wpool,
                tc.tile_pool(name="fin", bufs=2) as fpool,
                tc.tile_pool(name="hps", bufs=2, space="PSUM") as hpool,
                tc.tile_pool(name="sps", bufs=2, space="PSUM") as spool,
                tc.tile_pool(name="acc", bufs=2, space="PSUM") as apool,
            ):
                for g0 in range(0, NWIN if ablate != "p1" else 0, OB):
                  gnb = min(OB, NWIN - g0)
                  outb = fpool.tile([128, OB * HD], f32, tag="outb")
                  for k in range(gnb):
                    w = g0 + k
                    g = gpool.tile([128, kj * 128], u16, tag="g")
                    gv = g[:].unsqueeze(1)  # [128, 1, kj*128]
                    nc.gpsimd.dma_gather(
                        out_ap=gv[:, :, 0 : KL * 128], in_ap=xlo_d[:, :],
                        idxs_ap=il_all[:, w * KL * 8 : (w + 1) * KL * 8],
                        num_idxs=KL * 128, num_idxs_reg=KL * 128,
                        elem_size=128, transpose=True, single_packet=False,
                    )
                    nc.gpsimd.dma_gather(
                        out_ap=gv[:, :, KL * 128 : kj * 128], in_ap=xhi_d[:, :],
                        idxs_ap=ih_all[:, w * KH * 8 : (w + 1) * KH * 8],
                        num_idxs=KH * 128, num_idxs_reg=KH * 128,
                        elem_size=128, transpose=True, single_packet=False,
                    )
                    gb = g[:].bitcast(bf16)
                    adw = adst_all[:, w * H : (w + 1) * H]
                    drel_b = drel_all[:, w * kj : (w + 1) * kj]

                    if ablate == "p1g":
                        nc.vector.memset(outb[:, k * HD : (k + 1) * HD], 0.0)
                        nc.vector.tensor_copy(
                            outb[0:IN_DIM, k * HD : k * HD + HD],
                            gb[0:IN_DIM, 0:HD],
                        )
                        if k == gnb - 1:
                            nc.sync.dma_start(
                                out_d[g0 * 128 : (g0 + gnb) * 128, :].rearrange(
                                    "(k p) c -> p k c", p=128
                                ),
                                outb[:, 0 : gnb * HD].rearrange(
                                    "p (k c) -> p k c", c=HD
                                ),
                            )
                        continue

                    accdns = apool.tile([128, WCOLS], f32, tag="accdns")
                    for ch in range(nch):
                        s0 = ch * SUBS
                        ns = min(SUBS, kj - s0)
                        # per-edge projection: hps[:, s*512 : s*512+260] =
                        #   [h | a_src] of subtile s (bf16 matmul, f32 psum)
                        hps = hpool.tile([128, SUBS * CSTRIDE], f32, tag="hps")
                        for s in range(ns):
                            nc.tensor.matmul(
                                hps[:, s * CSTRIDE : s * CSTRIDE + WCOLS],
                                lhsT=gb[0:IN_DIM, (s0 + s) * 128 : (s0 + s + 1) * 128],
                                rhs=WAb[:],
                                start=True, stop=True,
                            )
                        # one-hot[e, s, p] = (dstrel[e, s] == p), bf16
                        oh = wpool.tile([128, SUBS * 128], bf16, tag="oh")
                        nc.vector.tensor_tensor(
                            out=oh[:, 0 : ns * 128].rearrange("p (s e) -> p s e", s=ns),
                            in0=drel_b[:, s0 : s0 + ns].unsqueeze(-1).to_broadcast(
                                [128, ns, 128]
                            ),
                            in1=iota_f[:].unsqueeze(1).to_broadcast([128, ns, 128]),
                            op=EQ,
                        )
                        # transposed one-hot (for a_dst expansion) + a_dst matmul
                        # (adx lands in unused hps columns ADX0:ADX0+H per slot
                        # -- no extra PSUM bank needed)
                        ohts = spool.tile([128, SUBS * 128], bf16, tag="ohts")
                        for s in range(ns):
                            nc.tensor.transpose(
                                ohts[:, s * 128 : (s + 1) * 128],
                                oh[:, s * 128 : (s + 1) * 128],
                                ident[:],
                            )
                        ohT = wpool.tile([128, SUBS * 128], bf16, tag="ohT")
                        nc.scalar.copy(ohT[:, 0 : ns * 128], ohts[:, 0 : ns * 128])
                        for s in range(ns):
                            nc.tensor.matmul(
                                hps[:, s * CSTRIDE + ADX0 : s * CSTRIDE + ADX0 + H],
                                lhsT=ohT[:, s * 128 : (s + 1) * 128],
                                rhs=adw,
                                start=True, stop=True,
                            )
                        # score = a_src + a_dst -> leaky_relu -> exp (bf16)
                        hv = hps[:].rearrange("p (s c) -> p s c", c=CSTRIDE)
                        sc = wpool.tile([128, SUBS * H], f32, tag="sc")
                        nc.vector.reduce_sum(
                            out=sc[:, 0 : ns * H].rearrange("p (s h) -> p s h", h=H),
                            in_=hv[:, 0:ns, HD : HD + 2 * H].rearrange(
                                "p s (a h) -> p s h a", a=2
                            ),
                            axis=X,
                        )
                        lr = wpool.tile([128, SUBS * H], f32, tag="lr")
                        nc.vector.scalar_tensor_tensor(
                            out=lr[:, 0 : ns * H], in0=sc[:, 0 : ns * H],
                            scalar=NEG_SLOPE, in1=sc[:, 0 : ns * H],
                            op0=MULT, op1=MAX,
                        )
                        ex = wpool.tile([128, SUBS * H], bf16, tag="ex")
                        nc.scalar.activation(
                            ex[:, 0 : ns * H], lr[:, 0 : ns * H],
                            mybir.ActivationFunctionType.Exp,
                        )
                        # msg[:, s, 0:HD] = h_s * e ; msg[:, s, HD:HD+H] = e
                        # (mul reads ex directly; the e-column copy runs in
                        # parallel, off the critical path)
                        msg = wpool.tile([128, SUBS * WCOLS], bf16, tag="msg")
                        mv = msg[:].rearrange("p (s c) -> p s c", s=SUBS)
                        nc.vector.tensor_copy(
                            mv[:, 0:ns, HD : HD + H],
                            ex[:, 0 : ns * H].rearrange("p (s h) -> p s h", h=H),
                        )
                        nc.vector.tensor_mul(
                            mv[:, 0:ns, 0:HD].rearrange("p s (h d) -> p s h d", d=D),
                            hv[:, 0:ns, 0:HD].rearrange("p s (h d) -> p s h d", d=D),
                            ex[:, 0 : ns * H]
                            .rearrange("p (s h) -> p s h", h=H)
                            .unsqueeze(-1)
                            .to_broadcast([128, ns, H, D]),
                        )
                        # accumulate [sum(e*h) | sum(e)] over the window
                        for s in range(ns):
                            q = s0 + s
                            nc.tensor.matmul(
                                accdns[:],
                                lhsT=oh[:, s * 128 : (s + 1) * 128],
                                rhs=mv[:, s, :],
                                start=(q == 0),
                                stop=(q == kj - 1),
                            )
                    # finalize: out = acc / (dns + eps) + bias
                    dnse = fpool.tile([128, H], f32, tag="dnse")
                    nc.vector.tensor_scalar_add(dnse[:], accdns[:, HD : HD + H], EPS)
                    dnr = fpool.tile([128, H], f32, tag="dnr")
                    nc.vector.reciprocal(dnr[:], dnse[:])
                    outw = outb[:, k * HD : (k + 1) * HD]
                    nc.vector.tensor_mul(
                        outw.rearrange("p (h d) -> p h d", d=D),
                        accdns[:, 0:HD].rearrange("p (h d) -> p h d", d=D),
                        dnr[:].unsqueeze(-1).to_broadcast([128, H, D]),
                    )
                    nc.vector.tensor_add(outw, outw, bias_bc[:])
                    if k == gnb - 1:
                        nc.sync.dma_start(
                            out_d[g0 * 128 : (g0 + gnb) * 128, :].rearrange(
                                "(k p) c -> p k c", p=128
                            ),
                            outb[:, 0 : gnb * HD].rearrange("p (k c) -> p k c", c=HD),
                        )
    nc.compile()
    # compile()'s late passes (act-table loads, hostgen rebases) can leave
    # >1-wait instructions behind; one more split pass clears them (the TRN2
    # ISA allows a single sem wait per compute instruction).
    nc.generate_event_semaphores()
    return nc


def _stage_inputs(x, W, att_src, att_dst, bias, ilow, ihigh, dstrel):
    x = np.asarray(x, dtype=np.float32)
    x_il = _interleave_x(x)
    x_lo = np.ascontiguousarray(x_il[:SPLIT])
    x_hi = np.ascontiguousarray(x_il[SPLIT:])
    asrc_row = np.ascontiguousarray(np.asarray(att_src, np.float32).reshape(1, HD))
    adst_row = np.ascontiguousarray(np.asarray(att_dst, np.float32).reshape(1, HD))
    bias_row = np.ascontiguousarray(np.asarray(bias, np.float32).reshape(1, HD))
    in_maps = []
    for c in range(NCORES):
        xdT = np.zeros((IN_DIM, WROWS), dtype=np.float32)
        xdT[:, :NPC] = x[c * NPC : (c + 1) * NPC].T
        in_maps.append(
            {
                "x_lo": x_lo,
                "x_hi": x_hi,
                "xdstT": np.ascontiguousarray(xdT),
                "W": np.asarray(W, np.float32),
                "att_src": asrc_row,
                "att_dst": adst_row,
                "bias": bias_row,
                "ilow": np.ascontiguousarray(
                    ilow[c].transpose(1, 0, 2).reshape(128, -1)
                ),
                "ihigh": np.ascontiguousarray(
                    ihigh[c].transpose(1, 0, 2).reshape(128, -1)
                ),
                "dstrel": np.ascontiguousarray(
                    dstrel[c].transpose(1, 0, 2).reshape(128, -1)
                ),
            }
        )
    return in_maps


def kernel(x, edge_index, W, att_src, att_dst, bias):
    global LAST_RESULTS
    from concourse.bass_utils import run_bass_kernel_spmd

    edge_index = np.asarray(edge_index)
    ilow, ihigh, dstrel, KL, KH = _prep_host(edge_index)
    nc = _build_program(KL, KH)
    in_maps = _stage_inputs(x, W, att_src, att_dst, bias, ilow, ihigh, dstrel)

    res = run_bass_kernel_spmd(nc, in_maps, list(range(NCORES)))
    LAST_RESULTS = res

    out = np.empty((N, HD), dtype=np.float32)
    for c in range(NCORES):
        out[c * NPC : (c + 1) * NPC] = res.results[c]["out"][:NPC]
    return out


# revision 38
# speedup vs baseline: 1.8344x; 1.6033x over previous
"""GAT layer (PyG-style, add_self_loops=True) on 8 Trainium2 NeuronCores.

Strategy: partition destination nodes (and their incident edges) across the 8
cores; each core owns a contiguous range of 6250 dst nodes (49 windows of 128).

No projection table. Per window of 128 dst nodes, two transposed dma_gathers
(lo/hi halves of the node range, int16-index limit) pull the raw 256-byte x
rows of all incident edges' sources straight out of HBM, TRANSPOSED at u16
granularity: the host pre-interleaves each x row's bytes as
[hi16(x_0)..hi16(x_63) | lo16(x_0)..lo16(x_63)], so gather partitions 0:64
hold truncated-bf16 features and serve directly as the matmul lhsT. Each
128-edge subtile then computes h|a_src = x_src @ [W | W@att_src^T] as ONE bf16
matmul (f32 PSUM), so there is no replicated 50k-row projection pass and no
38 MB table write at all.

Per-edge a_dst: one-hot(edge->dst slot) built on DVE in bf16, PE-transposed,
then a tiny matmul against the window's a_dst vector (phase-1b: 49 small
matmuls over the core's own dst shard). exp(leaky_relu(score)) on DVE+Act.
Segment-sum of [e*h | e] via PSUM-accumulated one-hot matmuls; final
out = acc/(denom+eps) + bias.  Softmax max-subtraction is skipped
(shift-invariant; scores are O(1) so exp cannot overflow).

Pad edges gather row 0 (finite) and carry dst-slot sentinel 500 => their
one-hot row is all zero, so they contribute to nothing. No dummy rows.

Host does only index/byte-space work (self-loop append, dst sort, windowing,
padding, int16 index wrapping, u16 byte interleave of x, x transpose/slice).
"""

import math

import numpy as np

N = 50000
IN_DIM = 64
H = 4
D = 64
HD = H * D  # 256
WCOLS = HD + H  # 260: per-edge matmul output h | a_src
NEG_SLOPE = 0.2
EPS = 1e-16
SENT = 500.0  # dst-slot sentinel for pad edges (one-hot row all zero)

NCORES = 8
NPC = N // NCORES  # 6250 dst nodes per core
NWIN = math.ceil(NPC / 128)  # 49 windows
WROWS = NWIN * 128  # 6272
SPLIT = 25088  # lo/hi x-table split (int16 gather index limit)
SUBS = 4  # edge subtiles per chunk (PSUM: hps 2 banks x2, spt 1 bank x2, acc 2)
USE_ACT_LRELU = False  # leaky_relu on Activation engine (CoreSim lacks Lrelu)

LAST_RESULTS = None  # BassKernelResults of the most recent run (for test.py)


def _wrap_idx(ids):
    """[n] int -> dma_gather wrapped layout [128, n/16] int16
    (idx i at [i%16, i//16], replicated across the 8 Q7 core groups)."""
    n = len(ids)
    w16 = ids.reshape(n // 16, 16).T.astype(np.int16)  # [16, n/16]
    return np.tile(w16, (8, 1))


def _interleave_x(x):
    """[N,64] f32 -> [N,128] u16 rows [hi16(x_0..63) | lo16(x_0..63)].
    After the u16-granularity transposed gather, partitions 0:64 hold the
    high halves = truncated-bf16 feature values."""
    xu = np.ascontiguousarray(x).view(np.uint16).reshape(-1, 64, 2)
    return np.ascontiguousarray(np.concatenate([xu[:, :, 1], xu[:, :, 0]], axis=1))


def _prep_host(edge_index):
    """Returns ilow  int16 [NCORES, NWIN, 128, KL*8]
               ihigh int16 [NCORES, NWIN, 128, KH*8]
               dstrel f32  [NCORES, NWIN, 128, KL+KH]  (slot or SENT)
               (KL, KH)"""
    src = np.concatenate([edge_index[0], np.arange(N, dtype=np.int64)]).astype(np.int64)
    dst = np.concatenate([edge_index[1], np.arange(N, dtype=np.int64)]).astype(np.int64)
    order = np.argsort(dst, kind="stable")
    src = src[order].astype(np.int32)
    dst = dst[order].astype(np.int32)

    bounds = [c * NPC + w * 128 for c in range(NCORES) for w in range(NWIN)]
    bounds.append(N)
    cuts = np.searchsorted(dst, np.asarray(bounds))

    lo_counts = np.zeros(NCORES * NWIN, np.int64)
    hi_counts = np.zeros(NCORES * NWIN, np.int64)
    for b in range(NCORES * NWIN):
        s = src[cuts[b] : cuts[b + 1]]
        lo_counts[b] = int((s < SPLIT).sum())
        hi_counts[b] = len(s) - lo_counts[b]
    KL = max(1, math.ceil(lo_counts.max() / 128))
    KH = max(1, math.ceil(hi_counts.max() / 128))
    kj = KL + KH

    ilow = np.zeros((NCORES, NWIN, KL * 128), np.int32)  # pad -> row 0 (finite)
    ihigh = np.zeros((NCORES, NWIN, KH * 128), np.int32)
    dstrel = np.full((NCORES, NWIN, 128, kj), SENT, np.float32)
    for c in range(NCORES):
        base = c * NPC
        for w in range(NWIN):
            b = c * NWIN + w
            s = src[cuts[b] : cuts[b + 1]]
            d = dst[cuts[b] : cuts[b + 1]] - base - w * 128
            m = s < SPLIT
            slo, dlo = s[m], d[m]
            shi, dhi = s[~m] - SPLIT, d[~m]
            # ascending source rows => HBM page locality in the gather
            o = np.argsort(slo, kind="stable")
            slo, dlo = slo[o], dlo[o]
            o = np.argsort(shi, kind="stable")
            shi, dhi = shi[o], dhi[o]
            ilow[c, w, : len(slo)] = slo
            ihigh[c, w, : len(shi)] = shi
            i = np.arange(len(slo))
            dstrel[c, w, i % 128, i // 128] = dlo
            i = np.arange(len(shi))
            dstrel[c, w, i % 128, KL + i // 128] = dhi
    ilow_w = np.zeros((NCORES, NWIN, 128, KL * 8), np.int16)
    ihigh_w = np.zeros((NCORES, NWIN, 128, KH * 8), np.int16)
    for c in range(NCORES):
        for w in range(NWIN):
            ilow_w[c, w] = _wrap_idx(ilow[c, w])
            ihigh_w[c, w] = _wrap_idx(ihigh[c, w])

    # one-hot (edge->slot) matrices in BOTH layouts as bf16 bit patterns
    # (0x3F80 = bf16 1.0), partition-major for contiguous per-window DMA:
    #   oh [p=e, w*kj*128 + s*128 + slot], ohT [p=slot, w*kj*128 + s*128 + e]
    one = np.uint16(0x3F80)
    slots = np.arange(128)
    oh_u = np.zeros((NCORES, 128, NWIN * kj * 128), np.uint16)
    ohT_u = np.zeros((NCORES, 128, NWIN * kj * 128), np.uint16)
    for c in range(NCORES):
        eq = dstrel[c][:, :, :, None] == slots  # [w, e, s, slot] bool
        oh_u[c] = (
            eq.transpose(1, 0, 2, 3).reshape(128, -1) * one
        )
        ohT_u[c] = (
            eq.transpose(3, 0, 2, 1).reshape(128, -1) * one
        )
    return ilow_w, ihigh_w, oh_u, ohT_u, KL, KH


def _build_program(KL, KH, ablate="full"):
    import concourse.bass as bass
    import concourse.bacc as bacc
    import concourse.tile as tile
    from concourse import mybir
    from concourse.masks import make_identity

    f32 = mybir.dt.float32
    bf16 = mybir.dt.bfloat16
    u16 = mybir.dt.uint16
    i16 = mybir.dt.int16
    i32 = mybir.dt.int32
    kj = KL + KH

    nc = bacc.Bacc(None, target_bir_lowering=False)

    xlo_d = nc.dram_tensor("x_lo", [SPLIT, 128], u16, kind="ExternalInput")
    xhi_d = nc.dram_tensor("x_hi", [N - SPLIT, 128], u16, kind="ExternalInput")
    xdT_d = nc.dram_tensor("xdstT", [IN_DIM, WROWS], f32, kind="ExternalInput")
    W_d = nc.dram_tensor("W", [IN_DIM, HD], f32, kind="ExternalInput")
    asrc_d = nc.dram_tensor("att_src", [1, HD], f32, kind="ExternalInput")
    adst_d = nc.dram_tensor("att_dst", [1, HD], f32, kind="ExternalInput")
    bias_d = nc.dram_tensor("bias", [1, HD], f32, kind="ExternalInput")
    # partition-major on host so the one-shot preload DMA is contiguous per
    # partition (128 large descriptors, not NWIN*128 small ones)
    il_d = nc.dram_tensor("ilow", [128, NWIN * KL * 8], i16, kind="ExternalInput")
    ih_d = nc.dram_tensor("ihigh", [128, NWIN * KH * 8], i16, kind="ExternalInput")
    # host-built one-hot matrices (bf16 bit patterns), both layouts
    oh_d = nc.dram_tensor("oh", [128, NWIN * kj * 128], u16, kind="ExternalInput")
    ohT_d = nc.dram_tensor("ohT", [128, NWIN * kj * 128], u16, kind="ExternalInput")
    out_d = nc.dram_tensor("out", [WROWS, HD], f32, kind="ExternalOutput")

    X = mybir.AxisListType.X
    EQ = mybir.AluOpType.is_equal
    MULT = mybir.AluOpType.mult
    MAX = mybir.AluOpType.max

    with tile.TileContext(nc) as tc:
        with tc.tile_pool(name="const", bufs=1) as cpool:
            spsum_cm = tc.tile_pool(name="setup_psum", bufs=1, space="PSUM")
            spsum = spsum_cm.__enter__()
            ones = cpool.tile([1, 128], f32)
            nc.vector.memset(ones[:], 1.0)

            # WA = [W | Wsrc], Wsrc[k,h] = sum_d W[k,h*D+d]*att_src[h,d];
            # Wdst likewise (kept f32 for the per-window a_dst matmuls).
            wa_tmp = cpool.tile([IN_DIM, WCOLS], f32)
            nc.vector.memset(wa_tmp[:], 0.0)
            nc.sync.dma_start(wa_tmp[:, 0:HD], W_d[:, :])
            wdst = cpool.tile([IN_DIM, H], f32)
            att_s_raw = cpool.tile([1, HD], f32)
            nc.sync.dma_start(att_s_raw[:], asrc_d[:, :])
            att_t_raw = cpool.tile([1, HD], f32)
            nc.sync.dma_start(att_t_raw[:], adst_d[:, :])
            att_s = cpool.tile([1, HD], f32)
            nc.vector.tensor_copy(att_s[:], att_s_raw[:])
            att_t = cpool.tile([1, HD], f32)
            nc.vector.tensor_copy(att_t[:], att_t_raw[:])
            for att_tile, dst_ap in ((att_s, wa_tmp[:, HD : HD + H]), (att_t, wdst[:, :])):
                attb = spsum.tile([IN_DIM, HD], f32, tag="attb")
                nc.tensor.matmul(
                    attb[:], lhsT=ones[:1, 0:IN_DIM], rhs=att_tile[:],
                    start=True, stop=True,
                )
                tmp = cpool.tile([IN_DIM, HD], f32, tag="tmp")
                nc.vector.tensor_mul(tmp[:], wa_tmp[:, 0:HD], attb[:])
                nc.vector.reduce_sum(
                    out=dst_ap,
                    in_=tmp[:].rearrange("k (h d) -> k h d", d=D),
                    axis=X,
                )
            WAb = cpool.tile([IN_DIM, WCOLS], bf16)
            nc.vector.tensor_copy(WAb[:], wa_tmp[:])

            bias_raw = cpool.tile([1, HD], f32)
            nc.sync.dma_start(bias_raw[:], bias_d[:, :])
            bias_sb = cpool.tile([1, HD], f32)
            nc.vector.tensor_copy(bias_sb[:], bias_raw[:])
            bb = spsum.tile([128, HD], f32)
            nc.tensor.matmul(bb[:], lhsT=ones[:1, :], rhs=bias_sb[:], start=True, stop=True)
            bias_bc = cpool.tile([128, HD], f32)
            nc.scalar.copy(bias_bc[:], bb[:])

            # phase 1b: a_dst for the core's own dst shard, bf16 [128, NWIN*H]
            adst_all = cpool.tile([128, NWIN * H], bf16)

            # preloaded per-window index data (batched DMAs -- per-DMA fixed
            # cost on HW is ~2us, so per-window loads are expensive)
            il_all = cpool.tile([128, NWIN * KL * 8], i16)
            nc.sync.dma_start(il_all[:], il_d[:, :])
            ih_all = cpool.tile([128, NWIN * KH * 8], i16)
            nc.sync.dma_start(ih_all[:], ih_d[:, :])

            spsum_cm.__exit__(None, None, None)  # free setup PSUM banks

            P1B = 8  # windows per psum batch
            with (
                tc.tile_pool(name="p1", bufs=1) as p1,
                tc.tile_pool(name="p1ps", bufs=2, space="PSUM") as p1ps,
            ):
                xd = p1.tile([IN_DIM, WROWS], f32, tag="xdr")
                nc.sync.dma_start(xd[:], xdT_d[:, :])
                for w0 in range(0, NWIN, P1B):
                    nb = min(P1B, NWIN - w0)
                    adp = p1ps.tile([128, P1B * H], f32, tag="adp")
                    for k in range(nb):
                        w = w0 + k
                        nc.tensor.matmul(
                            adp[:, k * H : (k + 1) * H],
                            lhsT=xd[:, w * 128 : (w + 1) * 128],
                            rhs=wdst[:], start=True, stop=True,
                        )
                    nc.vector.tensor_copy(
                        adst_all[:, w0 * H : (w0 + nb) * H], adp[:, 0 : nb * H]
                    )

            # ---------------- main loop: per-window aggregation ----------------
            if ablate == "p1":
                with tc.tile_pool(name="fin0", bufs=1) as f0:
                    zo = f0.tile([128, HD], f32)
                    nc.vector.memset(zo[:], 0.0)
                    for w in range(NWIN):
                        nc.sync.dma_start(out_d[w * 128 : (w + 1) * 128, :], zo[:])
            nch = math.ceil(kj / SUBS)
            OB = 7  # windows per output-write batch (49 = 7*7)
            LOOKAHEAD = 1  # chunks of PE h/asr/adx emitted ahead of acc
            with (
                tc.tile_pool(name="gat", bufs=3) as gpool,
                tc.tile_pool(name="ohp", bufs=2) as ohpool,
                tc.tile_pool(name="wrk", bufs=4) as wpool,
                tc.tile_pool(name="fin", bufs=2) as fpool,
                tc.tile_pool(name="hps", bufs=2, space="PSUM") as hpool,
                tc.tile_pool(name="sps", bufs=2, space="PSUM") as spool,
                tc.tile_pool(name="acc", bufs=2, space="PSUM") as apool,
            ):
                for g0 in range(0, NWIN if ablate != "p1" else 0, OB):
                  gnb = min(OB, NWIN - g0)
                  outb = fpool.tile([128, OB * HD], f32, tag="outb")
                  for k in range(gnb):
                    w = g0 + k
                    g = gpool.tile([128, kj * 128], u16, tag="g")
                    gv = g[:].unsqueeze(1)  # [128, 1, kj*128]
                    nc.gpsimd.dma_gather(
                        out_ap=gv[:, :, 0 : KL * 128], in_ap=xlo_d[:, :],
                        idxs_ap=il_all[:, w * KL * 8 : (w + 1) * KL * 8],
                        num_idxs=KL * 128, num_idxs_reg=KL * 128,
                        elem_size=128, transpose=True, single_packet=False,
                    )
                    nc.gpsimd.dma_gather(
                        out_ap=gv[:, :, KL * 128 : kj * 128], in_ap=xhi_d[:, :],
                        idxs_ap=ih_all[:, w * KH * 8 : (w + 1) * KH * 8],
                        num_idxs=KH * 128, num_idxs_reg=KH * 128,
                        elem_size=128, transpose=True, single_packet=False,
                    )
                    gb = g[:].bitcast(bf16)
                    adw = adst_all[:, w * H : (w + 1) * H]

                    if ablate == "p1g":
                        nc.vector.memset(outb[:, k * HD : (k + 1) * HD], 0.0)
                        nc.vector.tensor_copy(
                            outb[0:IN_DIM, k * HD : k * HD + HD],
                            gb[0:IN_DIM, 0:HD],
                        )
                        if k == gnb - 1:
                            nc.sync.dma_start(
                                out_d[g0 * 128 : (g0 + gnb) * 128, :].rearrange(
                                    "(k p) c -> p k c", p=128
                                ),
                                outb[:, 0 : gnb * HD].rearrange(
                                    "p (k c) -> p k c", c=HD
                                ),
                            )
                        continue

                    accdns = apool.tile([128, WCOLS], f32, tag="accdns")

                    # host-built one-hots, both layouts, streamed per window
                    oh_u = ohpool.tile([128, kj * 128], u16, tag="oh")
                    nc.sync.dma_start(
                        oh_u[:], oh_d[:, w * kj * 128 : (w + 1) * kj * 128]
                    )
                    ohT_u = ohpool.tile([128, kj * 128], u16, tag="ohT")
                    nc.sync.dma_start(
                        ohT_u[:], ohT_d[:, w * kj * 128 : (w + 1) * kj * 128]
                    )
                    oh_all = oh_u[:].bitcast(bf16)
                    ohT_all = ohT_u[:].bitcast(bf16)

                    # --- chunk phase, software-pipelined PE stream ---
                    # emit_h(c): projection h (1-bank psum) + score psum
                    # (a_src matmul accumulated with a_dst matmul)
                    def emit_h(c):
                        s0 = c * SUBS
                        ns = min(SUBS, kj - s0)
                        hps = hpool.tile([128, SUBS * HD], f32, tag="hps")
                        scps = spool.tile([128, SUBS * H], f32, tag="scps")
                        for s in range(ns):
                            sub = gb[
                                0:IN_DIM, (s0 + s) * 128 : (s0 + s + 1) * 128
                            ]
                            nc.tensor.matmul(
                                hps[:, s * HD : (s + 1) * HD],
                                lhsT=sub, rhs=WAb[:, 0:HD],
                                start=True, stop=True,
                            )
                            nc.tensor.matmul(
                                scps[:, s * H : (s + 1) * H],
                                lhsT=sub, rhs=WAb[:, HD : HD + H],
                                start=True, stop=False,
                            )
                            nc.tensor.matmul(
                                scps[:, s * H : (s + 1) * H],
                                lhsT=ohT_all[
                                    :, (s0 + s) * 128 : (s0 + s + 1) * 128
                                ],
                                rhs=adw,
                                start=False, stop=True,
                            )
                        return hps, scps, s0, ns

                    pending = [emit_h(c) for c in range(min(LOOKAHEAD, nch))]
                    for ch in range(nch):
                        if ch + LOOKAHEAD < nch:
                            pending.append(emit_h(ch + LOOKAHEAD))
                        hps, scps, s0, ns = pending[ch]
                        # msg[:, s, 0:HD] = h_s * e ; msg[:, s, HD:HD+H] = e
                        # exp writes the e columns of msg DIRECTLY (no copy op)
                        msg = wpool.tile([128, SUBS * WCOLS], bf16, tag="msg")
                        mv = msg[:].rearrange("p (s c) -> p s c", s=SUBS)
                        if USE_ACT_LRELU:
                            # leaky_relu on the Activation engine, then exp
                            lrx = wpool.tile([128, SUBS * H], f32, tag="lrx")
                            nc.scalar.activation(
                                lrx[:, 0 : ns * H], scps[:, 0 : ns * H],
                                mybir.ActivationFunctionType.Lrelu,
                                alpha=NEG_SLOPE,
                            )
                        else:
                            # leaky_relu = max(s, 0.2*s); each DVE op reads
                            # the score psum through a single input AP
                            sneg = wpool.tile([128, SUBS * H], f32, tag="sneg")
                            nc.vector.tensor_scalar_mul(
                                sneg[:, 0 : ns * H], scps[:, 0 : ns * H], NEG_SLOPE
                            )
                            lrx = wpool.tile([128, SUBS * H], f32, tag="lrx")
                            nc.vector.tensor_tensor(
                                out=lrx[:, 0 : ns * H],
                                in0=scps[:, 0 : ns * H],
                                in1=sneg[:, 0 : ns * H],
                                op=MAX,
                            )
                        nc.scalar.activation(
                            mv[:, 0:ns, HD : HD + H],
                            lrx[:, 0 : ns * H].rearrange("p (s h) -> p s h", h=H),
                            mybir.ActivationFunctionType.Exp,
                        )
                        exv = mv[:, 0:ns, HD : HD + H]
                        nc.vector.tensor_mul(
                            mv[:, 0:ns, 0:HD].rearrange("p s (h d) -> p s h d", d=D),
                            hps[:]
                            .rearrange("p (s c) -> p s c", c=HD)[:, 0:ns, :]
                            .rearrange("p s (h d) -> p s h d", d=D),
                            exv.unsqueeze(-1).to_broadcast([128, ns, H, D]),
                        )
                        # accumulate [sum(e*h) | sum(e)] over the window
                        for s in range(ns):
                            q = s0 + s
                            nc.tensor.matmul(
                                accdns[:],
                                lhsT=oh_all[:, (s0 + s) * 128 : (s0 + s + 1) * 128],
                                rhs=mv[:, s, :],
                                start=(q == 0),
                                stop=(q == kj - 1),
                            )
                    # finalize: out = acc / (dns + eps) + bias
                    dnse = fpool.tile([128, H], f32, tag="dnse")
                    nc.vector.tensor_scalar_add(dnse[:], accdns[:, HD : HD + H], EPS)
                    dnr = fpool.tile([128, H], f32, tag="dnr")
                    nc.vector.reciprocal(dnr[:], dnse[:])
                    outw = outb[:, k * HD : (k + 1) * HD]
                    nc.vector.tensor_mul(
                        outw.rearrange("p (h d) -> p h d", d=D),
                        accdns[:, 0:HD].rearrange("p (h d) -> p h d", d=D),
                        dnr[:].unsqueeze(-1).to_broadcast([128, H, D]),
                    )
                    nc.vector.tensor_add(outw, outw, bias_bc[:])
                    if k == gnb - 1:
                        nc.sync.dma_start(
                            out_d[g0 * 128 : (g0 + gnb) * 128, :].rearrange(
                                "(k p) c -> p k c", p=128
                            ),
                            outb[:, 0 : gnb * HD].rearrange("p (k c) -> p k c", c=HD),
                        )
    nc.compile()
    # compile()'s late passes (act-table loads, hostgen rebases) can leave
    # >1-wait instructions behind; one more split pass clears them (the TRN2
    # ISA allows a single sem wait per compute instruction).
    nc.generate_event_semaphores()
    return nc


def _stage_inputs(x, W, att_src, att_dst, bias, ilow, ihigh, oh_u, ohT_u):
    x = np.asarray(x, dtype=np.float32)
    x_il = _interleave_x(x)
    x_lo = np.ascontiguousarray(x_il[:SPLIT])
    x_hi = np.ascontiguousarray(x_il[SPLIT:])
    asrc_row = np.ascontiguousarray(np.asarray(att_src, np.float32).reshape(1, HD))
    adst_row = np.ascontiguousarray(np.asarray(att_dst, np.float32).reshape(1, HD))
    bias_row = np.ascontiguousarray(np.asarray(bias, np.float32).reshape(1, HD))
    in_maps = []
    for c in range(NCORES):
        xdT = np.zeros((IN_DIM, WROWS), dtype=np.float32)
        xdT[:, :NPC] = x[c * NPC : (c + 1) * NPC].T
        in_maps.append(
            {
                "x_lo": x_lo,
                "x_hi": x_hi,
                "xdstT": np.ascontiguousarray(xdT),
                "W": np.asarray(W, np.float32),
                "att_src": asrc_row,
                "att_dst": adst_row,
                "bias": bias_row,
                "ilow": np.ascontiguousarray(
                    ilow[c].transpose(1, 0, 2).reshape(128, -1)
                ),
                "ihigh": np.ascontiguousarray(
                    ihigh[c].transpose(1, 0, 2).reshape(128, -1)
                ),
                "oh": np.ascontiguousarray(oh_u[c]),
                "ohT": np.ascontiguousarray(ohT_u[c]),
            }
        )
    return in_maps


def kernel(x, edge_index, W, att_src, att_dst, bias):
    global LAST_RESULTS
    from concourse.bass_utils import run_bass_kernel_spmd

    edge_index = np.asarray(edge_index)
    ilow, ihigh, oh_u, ohT_u, KL, KH = _prep_host(edge_index)
    nc = _build_program(KL, KH)
    in_maps = _stage_inputs(x, W, att_src, att_dst, bias, ilow, ihigh, oh_u, ohT_u)

    res = run_bass_kernel_spmd(nc, in_maps, list(range(NCORES)))
    LAST_RESULTS = res

    out = np.empty((N, HD), dtype=np.float32)
    for c in range(NCORES):
        out[c * NPC : (c + 1) * NPC] = res.results[c]["out"][:NPC]
    return out
